# revision 16
# baseline (speedup 1.0000x reference)
"""Autoregressive GRU on 8 TRN2 NeuronCores.

Data-parallel: batch B=512 is split as 64 rows per core; the small GRU
weights are replicated and the T=128 sequential loop runs locally per core.

Key algebra (Keras GRU, reset_after=True, gate order [z, r, h]):
  step 0:  inp = 0, h = x  ->  gx = b[0], gh = x @ U + b[1]
  step t>=1: inp == h      ->  gx + gh uses (W + U) for the z and r gates
so per step we need ONE matmul against a host-prefused weight matrix:
  V  = [Wr+Ur | Uh | Wh | Wz+Uz]   (steps >= 1)   [D, 4D]
  V0 = [Ur   | Uh | 0  | Uz    ]   (step 0)       [D, 4D]
with PSUM bank layout [rpre | hh | xh | zpre], then
  r = sigmoid(rpre); hhat = tanh(xh + r*hh); z = sigmoid(zpre)
  h_new = hhat + z*(h - hhat)

Perf structure (what made this fast):
- float32r matmuls: full fp32 operand storage at bf16 stream rate (1
  cycle/row at N=512), so the weights carry no quantization error; only the
  bf16 recurrent state contributes (~1.2e-2 rel).
- One PSUM tile PER GATE BANK and per half: Tile's dependency tracking is
  tile-granular, so each consumer waits only for its own accumulation group.
- The serial tail (q -> tanh -> sigmoid(z) -> tt -> h_new -> transpose ->
  copy) is split into two D-halves held in SEPARATE tiles, so the halves
  pipeline across ACT/DVE/PE, and the next step's k0/k1 matmuls start as
  soon as the first half of hT is rebuilt (hT lives in two tiles).
- Scratch PE transposes mid-tail + a warm-up preamble keep the PE's HAM
  activity monitor at K=8/8 (otherwise every step's matmuls run at 1.2 GHz).
"""

import numpy as np
import ml_dtypes

B, D, T = 512, 512, 128
NCORES = 8
BLOC = B // NCORES  # 64
P = 128
KC = D // P  # 4 K-chunks
GW = 4 * D  # 2048 gate columns: [r | hh | xh | z]
H = D // 2  # 256: half of the hidden dim

_BF16 = ml_dtypes.bfloat16

# set by test harness to capture a profile; harmless when False
TRACE = False
TMPDIR = None
LAST = {}


def _prepare_weights(W, U, b):
    """Host-side fusion. Returns (V, V0, bias) in math layout."""
    Wz, Wr, Wh = W[:, :D], W[:, D : 2 * D], W[:, 2 * D :]
    Uz, Ur, Uh = U[:, :D], U[:, D : 2 * D], U[:, 2 * D :]
    V = np.concatenate([Wr + Ur, Uh, Wh, Wz + Uz], axis=1)  # [D, GW]
    V0 = np.concatenate([Ur, Uh, np.zeros_like(Wh), Uz], axis=1)
    b0, b1 = b[0], b[1]
    bias = np.concatenate(
        [b0[D : 2 * D] + b1[D : 2 * D], b1[2 * D :], b0[2 * D :], b0[:D] + b1[:D]]
    )  # [GW], order [r | hh | xh | z]
    return V, V0, bias


def _dev_layout(V):
    # V_dev[p, k*GW + j] = V[k*128 + p, j]
    return np.ascontiguousarray(
        V.reshape(KC, P, GW).transpose(1, 0, 2).reshape(P, KC * GW)
    )


_CACHE = {}


def _build(has_bias: bool):
    import concourse.mybir as mybir
    import concourse.tile as tile
    from concourse import bacc
    from concourse.masks import make_identity

    f32 = mybir.dt.float32
    f32r = mybir.dt.float32r
    bf16 = mybir.dt.bfloat16
    AF = mybir.ActivationFunctionType

    nc = bacc.Bacc(
        "TRN2", target_bir_lowering=False, debug=False, num_devices=NCORES
    )
    v0_d = nc.dram_tensor("v0", [P, KC * GW], f32r, kind="ExternalInput").ap()
    v_d = nc.dram_tensor("v", [P, KC * GW], f32r, kind="ExternalInput").ap()
    h0_d = nc.dram_tensor("h0", [BLOC, D], bf16, kind="ExternalInput").ap()
    h0T_d = nc.dram_tensor("h0T", [P, KC * BLOC], f32r, kind="ExternalInput").ap()
    if has_bias:
        bias_d = nc.dram_tensor("bias", [BLOC, GW], f32, kind="ExternalInput").ap()
    out_d = nc.dram_tensor("out", [BLOC, T, D], f32, kind="ExternalOutput").ap()

    # gate-bank column offsets inside V (per K-chunk block of width GW)
    OFF_R, OFF_HH, OFF_XH, OFF_Z = 0, 512, 1024, 1536

    with tile.TileContext(nc) as tc:
        with (
            tc.tile_pool(name="const", bufs=1) as cpool,
            tc.tile_pool(name="state", bufs=2) as spool,
            tc.tile_pool(name="work", bufs=3) as wpool,
            tc.tile_pool(name="outp", bufs=3) as opool,
            tc.tile_pool(name="gates", bufs=1, space="PSUM") as gpool,
            tc.tile_pool(name="trp", bufs=1, space="PSUM") as trpool,
            tc.tile_pool(name="warm", bufs=1, space="PSUM") as warmpool,
        ):
            v0_sb = cpool.tile([P, KC * GW], f32r, tag="v0")
            v_sb = cpool.tile([P, KC * GW], f32r, tag="v")
            ident = cpool.tile([BLOC, BLOC], bf16, tag="ident")
            nc.sync.dma_start(v0_sb[:], v0_d[:])
            make_identity(nc, ident[:])

            h_a = spool.tile([BLOC, H], bf16, tag="ha", name="h_a")
            h_b = spool.tile([BLOC, H], bf16, tag="hb", name="h_b")
            hT_a = spool.tile([P, 2 * BLOC], f32r, tag="hTa", name="hT_a")
            hT_b = spool.tile([P, 2 * BLOC], f32r, tag="hTb", name="hT_b")
            nc.sync.dma_start(h_a[:], h0_d[:, :H])
            nc.sync.dma_start(h_b[:], h0_d[:, H:])
            nc.sync.dma_start(hT_a[:], h0T_d[:, : 2 * BLOC])
            nc.sync.dma_start(hT_b[:], h0T_d[:, 2 * BLOC :])
            nc.sync.dma_start(v_sb[:], v_d[:])
            if has_bias:
                bias_sb = cpool.tile([BLOC, GW], f32, tag="bias")
                nc.sync.dma_start(bias_sb[:], bias_d[:])

            # PE warm-up: dense transpose work that depends only on the
            # locally-built identity (not on any DMA) flips the HAM clock
            # gate to K=8/8 while the weight DMAs are still in flight.
            wu = warmpool.tile([P, KC * BLOC], bf16, tag="warm", name="wu")
            for i in range(24):
                nc.tensor.matmul(
                    wu[:BLOC, (i % KC) * BLOC : (i % KC + 1) * BLOC],
                    ident[:],
                    ident[:],
                    is_transpose=True,
                    start=True,
                    stop=True,
                )

            def lhsT(k):
                if k < 2:
                    return hT_a[:, k * BLOC : (k + 1) * BLOC]
                return hT_b[:, (k - 2) * BLOC : (k - 1) * BLOC]

            for t in range(T):
                vsb = v0_sb if t == 0 else v_sb
                last = t == T - 1
                # per-bank PSUM tiles; z split in two half-banks so its
                # sigmoid halves can start independently
                g0 = gpool.tile([BLOC, 512], f32, tag="g0", name="g0")
                g1 = gpool.tile([BLOC, 512], f32, tag="g1", name="g1")
                g2 = gpool.tile([BLOC, 512], f32, tag="g2", name="g2")
                g3a = gpool.tile([BLOC, H], f32, tag="g3a", name="g3a")
                g3b = gpool.tile([BLOC, H], f32, tag="g3b", name="g3b")
                banks = [
                    (g0, OFF_R, 512),
                    (g1, OFF_HH, 512),
                    (g2, OFF_XH, 512),
                    (g3a, OFF_Z, H),
                    (g3b, OFF_Z + H, H),
                ]
                # k0/k1 first (they only need hT_a, which lands earlier)
                for ks, start, stop in (((0, 1), True, False), ((2, 3), False, True)):
                    for gt, off, width in banks:
                        for k in ks:
                            nc.tensor.matmul(
                                gt[:],
                                lhsT(k),
                                vsb[:, k * GW + off : k * GW + off + width],
                                start=start and k == ks[0],
                                stop=stop and k == ks[1],
                            )
                if has_bias:
                    nc.vector.tensor_add(g0[:], g0[:], bias_sb[:, 0:512])
                    nc.vector.tensor_add(g1[:], g1[:], bias_sb[:, 512:1024])
                    nc.vector.tensor_add(g2[:], g2[:], bias_sb[:, 1024:1536])
                    nc.vector.tensor_add(g3a[:], g3a[:], bias_sb[:, 1536:1792])
                    nc.vector.tensor_add(g3b[:], g3b[:], bias_sb[:, 1792:2048])

                # ACT stream: sig_r, tanh_a, sig_za, tanh_b, sig_zb
                r = wpool.tile([BLOC, D], bf16, tag="r", name="r")
                nc.scalar.activation(r[:], g0[:], AF.Sigmoid)
                # DVE stream: p, q_a, q_b, s_a, tt_a, f_a, s_b, cp_a, ...
                p = wpool.tile([BLOC, D], bf16, tag="p", name="p")
                nc.vector.tensor_mul(p[:], r[:], g1[:])
                q_a = wpool.tile([BLOC, H], bf16, tag="qa", name="q_a")
                nc.vector.tensor_add(q_a[:], p[:, :H], g2[:, :H])
                q_b = wpool.tile([BLOC, H], bf16, tag="qb", name="q_b")
                nc.vector.tensor_add(q_b[:], p[:, H:], g2[:, H:])

                hhat_a = wpool.tile([BLOC, H], bf16, tag="hha", name="hhat_a")
                nc.scalar.activation(hhat_a[:], q_a[:], AF.Tanh)
                z_a = wpool.tile([BLOC, H], bf16, tag="za", name="z_a")
                nc.scalar.activation(z_a[:], g3a[:], AF.Sigmoid)
                hhat_b = wpool.tile([BLOC, H], bf16, tag="hhb", name="hhat_b")
                nc.scalar.activation(hhat_b[:], q_b[:], AF.Tanh)
                z_b = wpool.tile([BLOC, H], bf16, tag="zb", name="z_b")
                nc.scalar.activation(z_b[:], g3b[:], AF.Sigmoid)

                if not last:
                    # PE filler mid-tail (HAM stays warm): scratch transposes
                    warm = warmpool.tile(
                        [P, KC * BLOC], bf16, tag="warm", name="warm"
                    )
                    for k in range(2):
                        nc.tensor.matmul(
                            warm[:, k * BLOC : (k + 1) * BLOC],
                            hhat_a[:, k * P : (k + 1) * P],
                            ident[:],
                            is_transpose=True,
                            start=True,
                            stop=True,
                        )

                s_a = wpool.tile([BLOC, H], bf16, tag="sa", name="s_a")
                nc.vector.tensor_sub(s_a[:], h_a[:], hhat_a[:])
                tt_a = wpool.tile([BLOC, H], bf16, tag="ta", name="tt_a")
                nc.vector.tensor_mul(tt_a[:], z_a[:], s_a[:])
                hn_a = spool.tile([BLOC, H], bf16, tag="ha", name="hn_a")
                nc.vector.tensor_add(hn_a[:], hhat_a[:], tt_a[:])

                if not last:
                    for k in range(2):
                        nc.tensor.matmul(
                            warm[:, (2 + k) * BLOC : (3 + k) * BLOC],
                            tt_a[:, k * P : (k + 1) * P],
                            ident[:],
                            is_transpose=True,
                            start=True,
                            stop=True,
                        )
                    trp_a = trpool.tile([P, 2 * BLOC], bf16, tag="tra", name="trp_a")
                    for k in range(2):
                        nc.tensor.matmul(
                            trp_a[:, k * BLOC : (k + 1) * BLOC],
                            hn_a[:, k * P : (k + 1) * P],
                            ident[:],
                            is_transpose=True,
                            start=True,
                            stop=True,
                        )

                s_b = wpool.tile([BLOC, H], bf16, tag="sb", name="s_b")
                nc.vector.tensor_sub(s_b[:], h_b[:], hhat_b[:])
                if not last:
                    hTa_new = spool.tile(
                        [P, 2 * BLOC], f32r, tag="hTa", name="hTa_new"
                    )
                    nc.vector.tensor_copy(hTa_new[:], trp_a[:])
                tt_b = wpool.tile([BLOC, H], bf16, tag="tb", name="tt_b")
                nc.vector.tensor_mul(tt_b[:], z_b[:], s_b[:])
                hn_b = spool.tile([BLOC, H], bf16, tag="hb", name="hn_b")
                nc.vector.tensor_add(hn_b[:], hhat_b[:], tt_b[:])
                if not last:
                    trp_b = trpool.tile([P, 2 * BLOC], bf16, tag="trb", name="trp_b")
                    for k in range(2):
                        nc.tensor.matmul(
                            trp_b[:, k * BLOC : (k + 1) * BLOC],
                            hn_b[:, k * P : (k + 1) * P],
                            ident[:],
                            is_transpose=True,
                            start=True,
                            stop=True,
                        )
                    hTb_new = spool.tile(
                        [P, 2 * BLOC], f32r, tag="hTb", name="hTb_new"
                    )
                    nc.vector.tensor_copy(hTb_new[:], trp_b[:])
                    hT_a, hT_b = hTa_new, hTb_new

                of_a = opool.tile([BLOC, H], f32, tag="ofa", name="of_a")
                nc.vector.tensor_copy(of_a[:], hn_a[:])
                nc.sync.dma_start(out_d[:, t, :H], of_a[:])
                of_b = opool.tile([BLOC, H], f32, tag="ofb", name="of_b")
                nc.vector.tensor_copy(of_b[:], hn_b[:])
                nc.sync.dma_start(out_d[:, t, H:], of_b[:])
                h_a, h_b = hn_a, hn_b

    nc.compile()
    return nc


def kernel(x, W, U, b):
    from concourse.bass_utils import run_bass_kernel_spmd

    x = np.asarray(x, dtype=np.float32)
    W = np.asarray(W, dtype=np.float32)
    U = np.asarray(U, dtype=np.float32)
    b = np.asarray(b, dtype=np.float32)

    V, V0, bias = _prepare_weights(W, U, b)
    has_bias = bool(np.any(bias != 0.0))
    v_dev = _dev_layout(V).astype(np.float32)
    v0_dev = _dev_layout(V0).astype(np.float32)

    key = ("gru", has_bias)
    if key not in _CACHE:
        _CACHE[key] = _build(has_bias)
    nc = _CACHE[key]

    in_maps = []
    for i in range(NCORES):
        xs = x[i * BLOC : (i + 1) * BLOC]  # [64, 512]
        m = {
            "v0": v0_dev,
            "v": v_dev,
            "h0": xs.astype(_BF16),
            "h0T": np.ascontiguousarray(
                xs.astype(_BF16)
                .astype(np.float32)
                .reshape(BLOC, KC, P)
                .transpose(2, 1, 0)
                .reshape(P, KC * BLOC)
            ),
        }
        if has_bias:
            m["bias"] = np.ascontiguousarray(
                np.broadcast_to(bias[None, :], (BLOC, GW))
            ).astype(np.float32)
        in_maps.append(m)

    res = run_bass_kernel_spmd(
        nc, in_maps, core_ids=list(range(NCORES)), trace=TRACE, tmpdir=TMPDIR
    )
    LAST["exec_time_ns"] = res.exec_time_ns
    LAST["results"] = res
    out = np.concatenate([res.results[i]["out"] for i in range(NCORES)], axis=0)
    return out.astype(np.float32)


# revision 20
# speedup vs baseline: 1.0453x; 1.0453x over previous
"""Autoregressive GRU on 8 TRN2 NeuronCores.

Data-parallel: batch B=512 is split as 64 rows per core; the small GRU
weights are replicated and the T=128 sequential loop runs locally per core.

Key algebra (Keras GRU, reset_after=True, gate order [z, r, h]):
  step 0:  inp = 0, h = x  ->  gx = b[0], gh = x @ U + b[1]
  step t>=1: inp == h      ->  gx + gh uses (W + U) for the z and r gates
so per step we need ONE matmul against a host-prefused weight matrix:
  V  = [Wr+Ur | Uh | Wh | Wz+Uz]   (steps >= 1)   [D, 4D]
  V0 = [Ur   | Uh | 0  | Uz    ]   (step 0)       [D, 4D]
with PSUM bank layout [rpre | hh | xh | zpre], then
  r = sigmoid(rpre); hhat = tanh(xh + r*hh); z = sigmoid(zpre)
  h_new = hhat + z*(h - hhat)

Perf structure (what made this fast):
- float32r matmuls: fp32 operand storage at bf16 stream rate (1 cycle/row at
  N=512), so weights carry no quantization error; only the bf16 recurrent
  state contributes.
- One PSUM tile PER GATE BANK: Tile's dependency tracking is tile-granular,
  so each consumer waits only on its own 4-matmul accumulation group.
- q = xh + r*hh is finished ON THE PE: after the xh bank's K-chunks, one
  extra accumulating matmul adds p = r*hh via an identity stationary
  operand (out += I.T @ p). tanh then reads the PSUM bank directly; the
  680 ns PSUM-sourced DVE add disappears from the critical chain.
- hT (the next step's stationary operand) is built by PE "transposes"
  expressed as regular matmuls against an identity moving operand
  (out = lhsT.T @ I), because regular matmuls ACCUMULATE in PSUM:
      trpT  = hhat^T   (start)  ... runs mid-tail, keeps HAM warm
      trpT += tt^T     (accumulate)
  so h_new^T = (hhat + z*(h-hhat))^T is ready one DVE-copy after tt, with
  the h_new add and the output copy running off-chain in parallel.
- A warm-up preamble of identity transposes (no DMA dependence) flips the
  PE clock gate to K=8/8 before step 0.
"""

import numpy as np
import ml_dtypes

B, D, T = 512, 512, 128
NCORES = 8
BLOC = B // NCORES  # 64
P = 128
KC = D // P  # 4 K-chunks
GW = 4 * D  # 2048 gate columns: [r | hh | xh | z]

_BF16 = ml_dtypes.bfloat16

# set by test harness to capture a profile; harmless when False
TRACE = False
TMPDIR = None
LAST = {}


def _prepare_weights(W, U, b):
    """Host-side fusion. Returns (V, V0, bias) in math layout."""
    Wz, Wr, Wh = W[:, :D], W[:, D : 2 * D], W[:, 2 * D :]
    Uz, Ur, Uh = U[:, :D], U[:, D : 2 * D], U[:, 2 * D :]
    V = np.concatenate([Wr + Ur, Uh, Wh, Wz + Uz], axis=1)  # [D, GW]
    V0 = np.concatenate([Ur, Uh, np.zeros_like(Wh), Uz], axis=1)
    b0, b1 = b[0], b[1]
    bias = np.concatenate(
        [b0[D : 2 * D] + b1[D : 2 * D], b1[2 * D :], b0[2 * D :], b0[:D] + b1[:D]]
    )  # [GW], order [r | hh | xh | z]
    return V, V0, bias


def _dev_layout(V):
    # V_dev[p, k*GW + j] = V[k*128 + p, j]
    return np.ascontiguousarray(
        V.reshape(KC, P, GW).transpose(1, 0, 2).reshape(P, KC * GW)
    )


_CACHE = {}


def _build(has_bias: bool):
    import concourse.mybir as mybir
    import concourse.tile as tile
    from concourse import bacc
    from concourse.masks import make_identity

    f32 = mybir.dt.float32
    f32r = mybir.dt.float32r
    bf16 = mybir.dt.bfloat16
    AF = mybir.ActivationFunctionType

    nc = bacc.Bacc(
        "TRN2", target_bir_lowering=False, debug=False, num_devices=NCORES
    )
    v0_d = nc.dram_tensor("v0", [P, KC * GW], f32r, kind="ExternalInput").ap()
    v_d = nc.dram_tensor("v", [P, KC * GW], f32r, kind="ExternalInput").ap()
    h0_d = nc.dram_tensor("h0", [BLOC, D], bf16, kind="ExternalInput").ap()
    h0T_d = nc.dram_tensor("h0T", [P, KC * BLOC], f32r, kind="ExternalInput").ap()
    if has_bias:
        bias_d = nc.dram_tensor("bias", [BLOC, GW], f32, kind="ExternalInput").ap()
    out_d = nc.dram_tensor("out", [BLOC, T, D], f32, kind="ExternalOutput").ap()

    with tile.TileContext(nc) as tc:
        with (
            tc.tile_pool(name="const", bufs=1) as cpool,
            tc.tile_pool(name="state", bufs=2) as spool,
            tc.tile_pool(name="work", bufs=3) as wpool,
            tc.tile_pool(name="outp", bufs=3) as opool,
            tc.tile_pool(name="gates", bufs=1, space="PSUM") as gpool,
            tc.tile_pool(name="trp", bufs=1, space="PSUM") as trpool,
        ):
            v0_sb = cpool.tile([P, KC * GW], f32r, tag="v0")
            v_sb = cpool.tile([P, KC * GW], f32r, tag="v")
            ident = cpool.tile([BLOC, BLOC], bf16, tag="ident")
            nc.sync.dma_start(v0_sb[:], v0_d[:])
            make_identity(nc, ident[:])

            h = spool.tile([BLOC, D], bf16, tag="h")
            hT = spool.tile([P, KC * BLOC], f32r, tag="hT")
            nc.sync.dma_start(h[:], h0_d[:])
            nc.sync.dma_start(hT[:], h0T_d[:])
            nc.sync.dma_start(v_sb[:], v_d[:])
            if has_bias:
                bias_sb = cpool.tile([BLOC, GW], f32, tag="bias")
                nc.sync.dma_start(bias_sb[:], bias_d[:])

            # PE warm-up: dense transpose work that depends only on the
            # locally-built identity (not on any DMA) flips the HAM clock
            # gate to K=8/8 while the weight DMAs are still in flight.
            wu = trpool.tile([P, 4 * 512], f32, tag="trp", name="wu")
            for i in range(24):
                nc.tensor.matmul(
                    wu[:BLOC, (i % KC) * 512 : (i % KC) * 512 + BLOC],
                    ident[:],
                    ident[:],
                    start=True,
                    stop=True,
                )

            for t in range(T):
                vsb = v0_sb if t == 0 else v_sb
                last = t == T - 1
                # one PSUM tile per gate bank: [r | hh | xh | z]
                gb = [
                    gpool.tile([BLOC, 512], f32, tag=f"g{n}", name=f"g{n}")
                    for n in range(4)
                ]
                def bank_mms(n, stop=True):
                    for k in range(KC):
                        nc.tensor.matmul(
                            gb[n][:],
                            hT[:, k * BLOC : (k + 1) * BLOC],
                            vsb[:, k * GW + n * 512 : k * GW + (n + 1) * 512],
                            start=(k == 0),
                            stop=(k == KC - 1) and stop,
                        )
                    if has_bias:
                        nc.vector.tensor_add(
                            gb[n][:], gb[n][:], bias_sb[:, n * 512 : (n + 1) * 512]
                        )

                bank_mms(0)  # rpre
                r = wpool.tile([BLOC, D], bf16, tag="r", name="r")
                nc.scalar.activation(r[:], gb[0][:], AF.Sigmoid)
                bank_mms(1)  # hh
                p = wpool.tile([BLOC, D], bf16, tag="p", name="p")
                nc.vector.tensor_mul(p[:], r[:], gb[1][:])
                bank_mms(2, stop=False)  # xh
                # q = xh + p on the PE: accumulate p into the xh bank via an
                # identity stationary operand (out += I.T @ p)
                nc.tensor.matmul(gb[2][:], ident[:], p[:], start=False, stop=True)
                bank_mms(3)  # zpre
                hhat = wpool.tile([BLOC, D], bf16, tag="hhat", name="hhat")
                nc.scalar.activation(hhat[:], gb[2][:], AF.Tanh)

                if not last:
                    # trp = hhat^T, via regular matmuls (they accumulate);
                    # doubles as mid-tail PE activity for the HAM clock gate.
                    # Chunk k sits at column k*512 so it owns PSUM bank k and
                    # its start=True can't disturb the other chunks' bits.
                    trp = trpool.tile([P, 4 * 512], f32, tag="trp", name="trp")
                    for k in range(KC):
                        nc.tensor.matmul(
                            trp[:, k * 512 : k * 512 + BLOC],
                            hhat[:, k * P : (k + 1) * P],
                            ident[:],
                            start=True,
                            stop=False,
                        )

                s = wpool.tile([BLOC, D], bf16, tag="s", name="s")
                nc.vector.tensor_sub(s[:], h[:], hhat[:])
                z = wpool.tile([BLOC, D], bf16, tag="z", name="z")
                nc.scalar.activation(z[:], gb[3][:], AF.Sigmoid)
                tt = wpool.tile([BLOC, D], bf16, tag="t", name="tt")
                nc.vector.tensor_mul(tt[:], z[:], s[:])

                if not last:
                    # trp += tt^T  ->  trp = h_new^T
                    for k in range(KC):
                        nc.tensor.matmul(
                            trp[:, k * 512 : k * 512 + BLOC],
                            tt[:, k * P : (k + 1) * P],
                            ident[:],
                            start=False,
                            stop=True,
                        )
                    hT_new = spool.tile([P, KC * BLOC], f32r, tag="hT")
                    nc.vector.tensor_copy(
                        hT_new[:],
                        trp.rearrange("p (k w) -> p k w", w=512)[:, :, :BLOC],
                    )
                    hT = hT_new

                h_new = spool.tile([BLOC, D], bf16, tag="h")
                nc.vector.tensor_add(h_new[:], hhat[:], tt[:])
                of = opool.tile([BLOC, D], f32, tag="of", name="of")
                nc.vector.tensor_copy(of[:], h_new[:])
                nc.sync.dma_start(out_d[:, t, :], of[:])
                h = h_new

    nc.compile()
    return nc


def kernel(x, W, U, b):
    from concourse.bass_utils import run_bass_kernel_spmd

    x = np.asarray(x, dtype=np.float32)
    W = np.asarray(W, dtype=np.float32)
    U = np.asarray(U, dtype=np.float32)
    b = np.asarray(b, dtype=np.float32)

    V, V0, bias = _prepare_weights(W, U, b)
    has_bias = bool(np.any(bias != 0.0))
    v_dev = _dev_layout(V).astype(np.float32)
    v0_dev = _dev_layout(V0).astype(np.float32)

    key = ("gru", has_bias)
    if key not in _CACHE:
        _CACHE[key] = _build(has_bias)
    nc = _CACHE[key]

    in_maps = []
    for i in range(NCORES):
        xs = x[i * BLOC : (i + 1) * BLOC]  # [64, 512]
        m = {
            "v0": v0_dev,
            "v": v_dev,
            "h0": xs.astype(_BF16),
            "h0T": np.ascontiguousarray(
                xs.astype(_BF16)
                .astype(np.float32)
                .reshape(BLOC, KC, P)
                .transpose(2, 1, 0)
                .reshape(P, KC * BLOC)
            ),
        }
        if has_bias:
            m["bias"] = np.ascontiguousarray(
                np.broadcast_to(bias[None, :], (BLOC, GW))
            ).astype(np.float32)
        in_maps.append(m)

    res = run_bass_kernel_spmd(
        nc, in_maps, core_ids=list(range(NCORES)), trace=TRACE, tmpdir=TMPDIR
    )
    LAST["exec_time_ns"] = res.exec_time_ns
    LAST["results"] = res
    out = np.concatenate([res.results[i]["out"] for i in range(NCORES)], axis=0)
    return out.astype(np.float32)


# revision 21
# speedup vs baseline: 1.1219x; 1.0732x over previous
"""Autoregressive GRU on 8 TRN2 NeuronCores.

Data-parallel: batch B=512 is split as 64 rows per core; the small GRU
weights are replicated and the T=128 sequential loop runs locally per core.

Key algebra (Keras GRU, reset_after=True, gate order [z, r, h]):
  step 0:  inp = 0, h = x  ->  gx = b[0], gh = x @ U + b[1]
  step t>=1: inp == h      ->  gx + gh uses (W + U) for the z and r gates
so per step we need ONE matmul against a host-prefused weight matrix:
  V  = [Wr+Ur | Uh | Wh | Wz+Uz]   (steps >= 1)   [D, 4D]
  V0 = [Ur   | Uh | 0  | Uz    ]   (step 0)       [D, 4D]
with PSUM bank layout [rpre | hh | xh | zpre], then
  r = sigmoid(rpre); hhat = tanh(xh + r*hh); z = sigmoid(zpre)
  h_new = hhat + z*(h - hhat)

Perf structure (what made this fast):
- float32r matmuls: fp32 operand storage at bf16 stream rate (1 cycle/row at
  N=512), so weights carry no quantization error; only the bf16 recurrent
  state contributes.
- One PSUM tile PER GATE BANK: Tile's dependency tracking is tile-granular,
  so each consumer waits only on its own 4-matmul accumulation group.
- q = xh + r*hh is finished ON THE PE: after the xh bank's K-chunks, one
  extra accumulating matmul adds p = r*hh via an identity stationary
  operand (out += I.T @ p). tanh then reads the PSUM bank directly; the
  680 ns PSUM-sourced DVE add disappears from the critical chain.
- hT (the next step's stationary operand) is built by PE "transposes"
  expressed as regular matmuls against an identity moving operand
  (out = lhsT.T @ I), because regular matmuls ACCUMULATE in PSUM:
      trpT  = hhat^T   (start)  ... runs mid-tail, keeps HAM warm
      trpT += tt^T     (accumulate)
  so h_new^T = (hhat + z*(h-hhat))^T is ready one DVE-copy after tt, with
  the h_new add and the output copy running off-chain in parallel.
- A warm-up preamble of identity transposes (no DMA dependence) flips the
  PE clock gate to K=8/8 before step 0.
"""

import numpy as np
import ml_dtypes

B, D, T = 512, 512, 128
NCORES = 8
BLOC = B // NCORES  # 64
P = 128
KC = D // P  # 4 K-chunks
GW = 4 * D  # 2048 gate columns: [r | hh | xh | z]

_BF16 = ml_dtypes.bfloat16

# set by test harness to capture a profile; harmless when False
TRACE = False
TMPDIR = None
LAST = {}


def _prepare_weights(W, U, b):
    """Host-side fusion. Returns (V, V0, bias) in math layout."""
    Wz, Wr, Wh = W[:, :D], W[:, D : 2 * D], W[:, 2 * D :]
    Uz, Ur, Uh = U[:, :D], U[:, D : 2 * D], U[:, 2 * D :]
    V = np.concatenate([Wr + Ur, Uh, Wh, Wz + Uz], axis=1)  # [D, GW]
    V0 = np.concatenate([Ur, Uh, np.zeros_like(Wh), Uz], axis=1)
    b0, b1 = b[0], b[1]
    bias = np.concatenate(
        [b0[D : 2 * D] + b1[D : 2 * D], b1[2 * D :], b0[2 * D :], b0[:D] + b1[:D]]
    )  # [GW], order [r | hh | xh | z]
    return V, V0, bias


def _dev_layout(V):
    # V_dev[p, k*GW + j] = V[k*128 + p, j]
    return np.ascontiguousarray(
        V.reshape(KC, P, GW).transpose(1, 0, 2).reshape(P, KC * GW)
    )


_CACHE = {}


def _build(has_bias: bool):
    import concourse.mybir as mybir
    import concourse.tile as tile
    from concourse import bacc
    from concourse.masks import make_identity

    f32 = mybir.dt.float32
    f32r = mybir.dt.float32r
    bf16 = mybir.dt.bfloat16
    AF = mybir.ActivationFunctionType

    nc = bacc.Bacc(
        "TRN2", target_bir_lowering=False, debug=False, num_devices=NCORES
    )
    v0_d = nc.dram_tensor("v0", [P, KC * GW], f32r, kind="ExternalInput").ap()
    v_d = nc.dram_tensor("v", [P, KC * GW], f32r, kind="ExternalInput").ap()
    h0_d = nc.dram_tensor("h0", [BLOC, D], bf16, kind="ExternalInput").ap()
    h0T_d = nc.dram_tensor("h0T", [P, KC * BLOC], f32r, kind="ExternalInput").ap()
    if has_bias:
        bias_d = nc.dram_tensor("bias", [BLOC, GW], f32, kind="ExternalInput").ap()
    out_d = nc.dram_tensor("out", [BLOC, T, D], f32, kind="ExternalOutput").ap()

    with tile.TileContext(nc) as tc:
        with (
            tc.tile_pool(name="const", bufs=1) as cpool,
            tc.tile_pool(name="state", bufs=2) as spool,
            tc.tile_pool(name="work", bufs=3) as wpool,
            tc.tile_pool(name="outp", bufs=3) as opool,
            tc.tile_pool(name="gates", bufs=1, space="PSUM") as gpool,
            tc.tile_pool(name="trp", bufs=2, space="PSUM") as trpool,
            tc.tile_pool(name="warm", bufs=1, space="PSUM") as warmpool,
        ):
            v0_sb = cpool.tile([P, KC * GW], f32r, tag="v0")
            v_sb = cpool.tile([P, KC * GW], f32r, tag="v")
            ident = cpool.tile([BLOC, BLOC], bf16, tag="ident")
            nc.sync.dma_start(v0_sb[:], v0_d[:])
            make_identity(nc, ident[:])

            h = spool.tile([BLOC, D], bf16, tag="h")
            hT = spool.tile([P, KC * BLOC], f32r, tag="hT")
            nc.sync.dma_start(h[:], h0_d[:])
            nc.sync.dma_start(hT[:], h0T_d[:])
            nc.sync.dma_start(v_sb[:], v_d[:])
            if has_bias:
                bias_sb = cpool.tile([BLOC, GW], f32, tag="bias")
                nc.sync.dma_start(bias_sb[:], bias_d[:])

            # PE warm-up: dense transpose work that depends only on the
            # locally-built identity (not on any DMA) flips the HAM clock
            # gate to K=8/8 while the weight DMAs are still in flight.
            wu = trpool.tile([P, KC * BLOC], bf16, tag="trp", name="wu")
            for i in range(24):
                nc.tensor.matmul(
                    wu[:BLOC, (i % KC) * BLOC : (i % KC + 1) * BLOC],
                    ident[:],
                    ident[:],
                    is_transpose=True,
                    start=True,
                    stop=True,
                )

            for t in range(T):
                vsb = v0_sb if t == 0 else v_sb
                last = t == T - 1
                # one PSUM tile per gate bank: [r | hh | xh | z]
                gb = [
                    gpool.tile([BLOC, 512], f32, tag=f"g{n}", name=f"g{n}")
                    for n in range(4)
                ]
                def bank_mms(n, stop=True):
                    for k in range(KC):
                        nc.tensor.matmul(
                            gb[n][:],
                            hT[:, k * BLOC : (k + 1) * BLOC],
                            vsb[:, k * GW + n * 512 : k * GW + (n + 1) * 512],
                            start=(k == 0),
                            stop=(k == KC - 1) and stop,
                        )
                    if has_bias:
                        nc.vector.tensor_add(
                            gb[n][:], gb[n][:], bias_sb[:, n * 512 : (n + 1) * 512]
                        )

                bank_mms(0)  # rpre
                r = wpool.tile([BLOC, D], bf16, tag="r", name="r")
                nc.scalar.activation(r[:], gb[0][:], AF.Sigmoid)
                bank_mms(1)  # hh
                p = wpool.tile([BLOC, D], bf16, tag="p", name="p")
                nc.vector.tensor_mul(p[:], r[:], gb[1][:])
                bank_mms(2, stop=False)  # xh
                # q = xh + p on the PE: accumulate p into the xh bank via an
                # identity stationary operand (out += I.T @ p)
                nc.tensor.matmul(gb[2][:], ident[:], p[:], start=False, stop=True)
                bank_mms(3)  # zpre
                hhat = wpool.tile([BLOC, D], bf16, tag="hhat", name="hhat")
                nc.scalar.activation(hhat[:], gb[2][:], AF.Tanh)

                if not last:
                    # scratch transpose-mode ops (power-cheap, output unused):
                    # real PE activity mid-tail keeps the HAM clock gate warm
                    warm = warmpool.tile([P, KC * BLOC], bf16, tag="warm", name="warm")
                    for k in range(KC):
                        nc.tensor.matmul(
                            warm[:, k * BLOC : (k + 1) * BLOC],
                            hhat[:, k * P : (k + 1) * P],
                            ident[:],
                            is_transpose=True,
                            start=True,
                            stop=True,
                        )

                s = wpool.tile([BLOC, D], bf16, tag="s", name="s")
                nc.vector.tensor_sub(s[:], h[:], hhat[:])
                z = wpool.tile([BLOC, D], bf16, tag="z", name="z")
                nc.scalar.activation(z[:], gb[3][:], AF.Sigmoid)
                tt = wpool.tile([BLOC, D], bf16, tag="t", name="tt")
                nc.vector.tensor_mul(tt[:], z[:], s[:])

                h_new = spool.tile([BLOC, D], bf16, tag="h")
                nc.vector.tensor_add(h_new[:], hhat[:], tt[:])
                if not last:
                    trp = trpool.tile([P, KC * BLOC], bf16, tag="trp", name="trp")
                    for k in range(KC):
                        nc.tensor.matmul(
                            trp[:, k * BLOC : (k + 1) * BLOC],
                            h_new[:, k * P : (k + 1) * P],
                            ident[:],
                            is_transpose=True,
                            start=True,
                            stop=True,
                        )
                    hT_new = spool.tile([P, KC * BLOC], f32r, tag="hT")
                    nc.vector.tensor_copy(hT_new[:], trp[:])
                    hT = hT_new
                of = opool.tile([BLOC, D], f32, tag="of", name="of")
                nc.vector.tensor_copy(of[:], h_new[:])
                nc.sync.dma_start(out_d[:, t, :], of[:])
                h = h_new

    nc.compile()
    return nc


def kernel(x, W, U, b):
    from concourse.bass_utils import run_bass_kernel_spmd

    x = np.asarray(x, dtype=np.float32)
    W = np.asarray(W, dtype=np.float32)
    U = np.asarray(U, dtype=np.float32)
    b = np.asarray(b, dtype=np.float32)

    V, V0, bias = _prepare_weights(W, U, b)
    has_bias = bool(np.any(bias != 0.0))
    v_dev = _dev_layout(V).astype(np.float32)
    v0_dev = _dev_layout(V0).astype(np.float32)

    key = ("gru", has_bias)
    if key not in _CACHE:
        _CACHE[key] = _build(has_bias)
    nc = _CACHE[key]

    in_maps = []
    for i in range(NCORES):
        xs = x[i * BLOC : (i + 1) * BLOC]  # [64, 512]
        m = {
            "v0": v0_dev,
            "v": v_dev,
            "h0": xs.astype(_BF16),
            "h0T": np.ascontiguousarray(
                xs.astype(_BF16)
                .astype(np.float32)
                .reshape(BLOC, KC, P)
                .transpose(2, 1, 0)
                .reshape(P, KC * BLOC)
            ),
        }
        if has_bias:
            m["bias"] = np.ascontiguousarray(
                np.broadcast_to(bias[None, :], (BLOC, GW))
            ).astype(np.float32)
        in_maps.append(m)

    res = run_bass_kernel_spmd(
        nc, in_maps, core_ids=list(range(NCORES)), trace=TRACE, tmpdir=TMPDIR
    )
    LAST["exec_time_ns"] = res.exec_time_ns
    LAST["results"] = res
    out = np.concatenate([res.results[i]["out"] for i in range(NCORES)], axis=0)
    return out.astype(np.float32)


# revision 23
# speedup vs baseline: 1.2463x; 1.1109x over previous
"""Autoregressive GRU on 8 TRN2 NeuronCores.

Data-parallel: batch B=512 is split as 64 rows per core; the small GRU
weights are replicated and the T=128 sequential loop runs locally per core.

Key algebra (Keras GRU, reset_after=True, gate order [z, r, h]):
  step 0:  inp = 0, h = x  ->  gx = b[0], gh = x @ U + b[1]
  step t>=1: inp == h      ->  gx + gh uses (W + U) for the z and r gates
so per step we need ONE matmul against a host-prefused weight matrix:
  V  = [Wr+Ur | Uh | Wh | Wz+Uz]   (steps >= 1)   [D, 4D]
  V0 = [Ur   | Uh | 0  | Uz    ]   (step 0)       [D, 4D]
with PSUM bank layout [rpre | hh | xh | zpre], then
  r = sigmoid(rpre); hhat = tanh(xh + r*hh); z = sigmoid(zpre)
  h_new = hhat + z*(h - hhat)

Perf structure (what made this fast):
- float32r matmuls: fp32 operand storage at bf16 stream rate (1 cycle/row at
  N=512), so weights carry no quantization error; only the bf16 recurrent
  state contributes.
- One PSUM tile PER GATE BANK: Tile's dependency tracking is tile-granular,
  so each consumer waits only on its own 4-matmul accumulation group.
- q = xh + r*hh is finished ON THE PE: after the xh bank's K-chunks, one
  extra accumulating matmul adds p = r*hh via an identity stationary
  operand (out += I.T @ p). tanh then reads the PSUM bank directly; the
  680 ns PSUM-sourced DVE add disappears from the critical chain.
- hT (the next step's stationary operand) is built by PE "transposes"
  expressed as regular matmuls against an identity moving operand
  (out = lhsT.T @ I), because regular matmuls ACCUMULATE in PSUM:
      trpT  = hhat^T   (start)  ... runs mid-tail, keeps HAM warm
      trpT += tt^T     (accumulate)
  so h_new^T = (hhat + z*(h-hhat))^T is ready one DVE-copy after tt, with
  the h_new add and the output copy running off-chain in parallel.
- A warm-up preamble of identity transposes (no DMA dependence) flips the
  PE clock gate to K=8/8 before step 0.
"""

import numpy as np
import ml_dtypes

B, D, T = 512, 512, 128
NCORES = 8
BLOC = B // NCORES  # 64
P = 128
KC = D // P  # 4 K-chunks
GW = 4 * D  # 2048 gate columns: [r | hh | xh | z]

_BF16 = ml_dtypes.bfloat16

# set by test harness to capture a profile; harmless when False
TRACE = False
TMPDIR = None
LAST = {}


def _prepare_weights(W, U, b):
    """Host-side fusion. Returns (V, V0, bias) in math layout."""
    Wz, Wr, Wh = W[:, :D], W[:, D : 2 * D], W[:, 2 * D :]
    Uz, Ur, Uh = U[:, :D], U[:, D : 2 * D], U[:, 2 * D :]
    V = np.concatenate([Wr + Ur, Uh, Wh, Wz + Uz], axis=1)  # [D, GW]
    V0 = np.concatenate([Ur, Uh, np.zeros_like(Wh), Uz], axis=1)
    b0, b1 = b[0], b[1]
    bias = np.concatenate(
        [b0[D : 2 * D] + b1[D : 2 * D], b1[2 * D :], b0[2 * D :], b0[:D] + b1[:D]]
    )  # [GW], order [r | hh | xh | z]
    return V, V0, bias


def _dev_layout(V):
    # V_dev[p, k*GW + j] = V[k*128 + p, j]
    return np.ascontiguousarray(
        V.reshape(KC, P, GW).transpose(1, 0, 2).reshape(P, KC * GW)
    )


_CACHE = {}


def _build(has_bias: bool):
    import concourse.mybir as mybir
    import concourse.tile as tile
    from concourse import bacc
    from concourse.masks import make_identity

    f32 = mybir.dt.float32
    f32r = mybir.dt.float32r
    bf16 = mybir.dt.bfloat16
    AF = mybir.ActivationFunctionType

    nc = bacc.Bacc(
        "TRN2", target_bir_lowering=False, debug=False, num_devices=NCORES
    )
    v0_d = nc.dram_tensor("v0", [P, KC * GW], f32r, kind="ExternalInput").ap()
    v_d = nc.dram_tensor("v", [P, KC * GW], f32r, kind="ExternalInput").ap()
    h0_d = nc.dram_tensor("h0", [BLOC, D], bf16, kind="ExternalInput").ap()
    h0T_d = nc.dram_tensor("h0T", [P, KC * BLOC], f32r, kind="ExternalInput").ap()
    if has_bias:
        bias_d = nc.dram_tensor("bias", [BLOC, GW], f32, kind="ExternalInput").ap()
    out_d = nc.dram_tensor("out", [BLOC, T, D], f32, kind="ExternalOutput").ap()

    with tile.TileContext(nc) as tc:
        with (
            tc.tile_pool(name="const", bufs=1) as cpool,
            tc.tile_pool(name="state", bufs=2) as spool,
            tc.tile_pool(name="work", bufs=3) as wpool,
            tc.tile_pool(name="outp", bufs=3) as opool,
            tc.tile_pool(name="gates", bufs=1, space="PSUM") as gpool,
            tc.tile_pool(name="trp", bufs=2, space="PSUM") as trpool,
            tc.tile_pool(name="warm", bufs=1, space="PSUM") as warmpool,
        ):
            v0_sb = cpool.tile([P, KC * GW], f32r, tag="v0")
            v_sb = cpool.tile([P, KC * GW], f32r, tag="v")
            ident = cpool.tile([BLOC, BLOC], bf16, tag="ident")
            nc.sync.dma_start(v0_sb[:], v0_d[:])
            make_identity(nc, ident[:])

            h = spool.tile([BLOC, D], bf16, tag="h")
            hT = spool.tile([P, KC * BLOC], f32r, tag="hT")
            nc.sync.dma_start(h[:], h0_d[:])
            nc.sync.dma_start(hT[:], h0T_d[:])
            nc.sync.dma_start(v_sb[:], v_d[:])
            if has_bias:
                bias_sb = cpool.tile([BLOC, GW], f32, tag="bias")
                nc.sync.dma_start(bias_sb[:], bias_d[:])

            # PE warm-up: dense transpose work that depends only on the
            # locally-built identity (not on any DMA) flips the HAM clock
            # gate to K=8/8 while the weight DMAs are still in flight.
            wu = trpool.tile([P, KC * BLOC], bf16, tag="trp", name="wu")
            for i in range(24):
                nc.tensor.matmul(
                    wu[:BLOC, (i % KC) * BLOC : (i % KC + 1) * BLOC],
                    ident[:],
                    ident[:],
                    is_transpose=True,
                    start=True,
                    stop=True,
                )

            for t in range(T):
                vsb = v0_sb if t == 0 else v_sb
                last = t == T - 1
                # one PSUM tile per gate bank: [r | hh | xh | z]
                gb = [
                    gpool.tile([BLOC, 512], f32, tag=f"g{n}", name=f"g{n}")
                    for n in range(4)
                ]
                def bank_mms(n, stop=True):
                    for k in range(KC):
                        nc.tensor.matmul(
                            gb[n][:],
                            hT[:, k * BLOC : (k + 1) * BLOC],
                            vsb[:, k * GW + n * 512 : k * GW + (n + 1) * 512],
                            start=(k == 0),
                            stop=(k == KC - 1) and stop,
                        )
                    if has_bias:
                        nc.vector.tensor_add(
                            gb[n][:], gb[n][:], bias_sb[:, n * 512 : (n + 1) * 512]
                        )

                bank_mms(0)  # rpre
                r = wpool.tile([BLOC, D], bf16, tag="r", name="r")
                nc.scalar.activation(r[:], gb[0][:], AF.Sigmoid)
                bank_mms(1)  # hh
                p = wpool.tile([BLOC, D], bf16, tag="p", name="p")
                nc.vector.tensor_mul(p[:], r[:], gb[1][:])
                bank_mms(2)  # xh
                q = wpool.tile([BLOC, D], bf16, tag="q", name="q")
                nc.vector.tensor_add(q[:], p[:], gb[2][:])
                bank_mms(3)  # zpre
                hhat = wpool.tile([BLOC, D], bf16, tag="hhat", name="hhat")
                nc.scalar.activation(hhat[:], q[:], AF.Tanh)

                if not last:
                    # trpA = hhat^T: real mid-tail PE activity (keeps the HAM
                    # clock gate warm) that feeds the hT rebuild below
                    trpA = warmpool.tile(
                        [P, KC * BLOC], bf16, tag="warm", name="trpA"
                    )
                    for k in range(KC):
                        nc.tensor.matmul(
                            trpA[:, k * BLOC : (k + 1) * BLOC],
                            hhat[:, k * P : (k + 1) * P],
                            ident[:],
                            is_transpose=True,
                            start=True,
                            stop=True,
                        )

                s = wpool.tile([BLOC, D], bf16, tag="s", name="s")
                nc.vector.tensor_sub(s[:], h[:], hhat[:])
                z = wpool.tile([BLOC, D], bf16, tag="z", name="z")
                nc.scalar.activation(z[:], gb[3][:], AF.Sigmoid)
                tt = wpool.tile([BLOC, D], bf16, tag="t", name="tt")
                nc.vector.tensor_mul(tt[:], z[:], s[:])

                if not last:
                    # trpB = tt^T; then hT_new = trpA^ + trpB^ = h_new^T
                    # (transpose is linear), so the h_new add, the f32 output
                    # copy and the DMA all run OFF the recurrence chain
                    trpB = trpool.tile([P, KC * BLOC], bf16, tag="trp", name="trpB")
                    for k in range(KC):
                        nc.tensor.matmul(
                            trpB[:, k * BLOC : (k + 1) * BLOC],
                            tt[:, k * P : (k + 1) * P],
                            ident[:],
                            is_transpose=True,
                            start=True,
                            stop=True,
                        )
                    hT_new = spool.tile([P, KC * BLOC], f32r, tag="hT")
                    nc.vector.tensor_copy(hT_new[:], trpA[:])
                    nc.vector.tensor_add(hT_new[:], hT_new[:], trpB[:])
                    hT = hT_new

                h_new = spool.tile([BLOC, D], bf16, tag="h")
                nc.vector.tensor_add(h_new[:], hhat[:], tt[:])
                of = opool.tile([BLOC, D], f32, tag="of", name="of")
                nc.vector.tensor_copy(of[:], h_new[:])
                nc.sync.dma_start(out_d[:, t, :], of[:])
                h = h_new

    nc.compile()
    return nc


def kernel(x, W, U, b):
    from concourse.bass_utils import run_bass_kernel_spmd

    x = np.asarray(x, dtype=np.float32)
    W = np.asarray(W, dtype=np.float32)
    U = np.asarray(U, dtype=np.float32)
    b = np.asarray(b, dtype=np.float32)

    V, V0, bias = _prepare_weights(W, U, b)
    has_bias = bool(np.any(bias != 0.0))
    v_dev = _dev_layout(V).astype(np.float32)
    v0_dev = _dev_layout(V0).astype(np.float32)

    key = ("gru", has_bias)
    if key not in _CACHE:
        _CACHE[key] = _build(has_bias)
    nc = _CACHE[key]

    in_maps = []
    for i in range(NCORES):
        xs = x[i * BLOC : (i + 1) * BLOC]  # [64, 512]
        m = {
            "v0": v0_dev,
            "v": v_dev,
            "h0": xs.astype(_BF16),
            "h0T": np.ascontiguousarray(
                xs.astype(_BF16)
                .astype(np.float32)
                .reshape(BLOC, KC, P)
                .transpose(2, 1, 0)
                .reshape(P, KC * BLOC)
            ),
        }
        if has_bias:
            m["bias"] = np.ascontiguousarray(
                np.broadcast_to(bias[None, :], (BLOC, GW))
            ).astype(np.float32)
        in_maps.append(m)

    res = run_bass_kernel_spmd(
        nc, in_maps, core_ids=list(range(NCORES)), trace=TRACE, tmpdir=TMPDIR
    )
    LAST["exec_time_ns"] = res.exec_time_ns
    LAST["results"] = res
    out = np.concatenate([res.results[i]["out"] for i in range(NCORES)], axis=0)
    return out.astype(np.float32)
